# revision 24
# baseline (speedup 1.0000x reference)
"""LoRA layer kernel for Trainium2, 8-core data-parallel.

out = x @ W.T + 2.0 * ((x @ B) @ A)
  x: (4, 4096, 4096) f32, W: (4096, 4096), A: (16, 4096), B: (4096, 16)

v2 strategy:
  - Host folds LoRA into the weight: W' = W.T + 2*(B@A)  ([in,out]),
    so the device runs a single pure GEMM out = x @ W'.
  - bf16 operands (same PE rate as fp32r at 1 col/cycle, half the DMA
    traffic; fp32 PSUM accumulation keeps rel err ~3e-3).
  - Rows sharded across 8 cores (2048 rows each). Per core the x panel
    ([4096 K, 2048 M] bf16 = 128 KB/partition) is fully SBUF-resident,
    so W' streams from HBM exactly once (32 MB bf16).
  - W-stationary / x-moving: out.T[o,m] = sum_k W'[k,o]*x[k,m]. Each
    pass covers a 256-wide o-pair: 8 PSUM banks = 2 o-tiles x 4
    m-chunks, accumulated over all 32 k-tiles (K-contiguous keeps the
    PE HAM-warm). 4 consecutive matmuls share one stationary W tile.
  - Output produced transposed ([4096 o, 2048 m] per core); host
    transposes back when gathering.
"""

import sys

if "/opt/trn_rl_repo" not in sys.path:
    sys.path.insert(0, "/opt/trn_rl_repo")

import os

import numpy as np
import ml_dtypes

WARMUP = os.environ.get("K_WARMUP", "1") == "1"
XQ2 = os.environ.get("K_XQ2", "1") == "1"
OALT = os.environ.get("K_OALT", "1") == "1"
SPLIT = os.environ.get("K_SPLIT", "1") == "1"

import concourse.bass as bass
import concourse.mybir as mybir
import concourse.tile as tile

N_CORES = 8
D = 4096
RANK = 16
ROWS_TOTAL = 4 * 4096                   # 16384
ROWS_PER_CORE = ROWS_TOTAL // N_CORES   # 2048
P = 128
NKT = D // P                            # 32 k-tiles
NPASS = 16                              # o-pairs of 256
OPW = 256                               # o columns per pass
MC = 512                                # moving m-chunk width
NMC = ROWS_PER_CORE // MC               # 4 m-chunks
NWQ = 4                                 # W quarter-panels per pass

F32 = mybir.dt.float32
BF16 = mybir.dt.bfloat16
BF = ml_dtypes.bfloat16


def split_wide_waits(nc, max_waits=1):
    """walrus in this container rejects >1 sync wait per instruction;
    move excess waits onto preceding same-engine NoOps."""
    n_split = 0
    for f in nc.m.functions:
        for bb in f.blocks:
            new_insts = []
            for inst in bb.instructions:
                si = getattr(inst, "sync_info", None)
                if si is not None and si.on_wait and len(si.on_wait) > max_waits:
                    waits = list(si.on_wait)
                    keep = waits[-max_waits:]
                    extra = waits[:-max_waits]
                    for i in range(0, len(extra), max_waits):
                        chunk = extra[i:i + max_waits]
                        nop = mybir.InstNoOp(
                            name=f"{inst.name}_wsplit{i}",
                            sync_info=mybir.SyncInfo(on_wait=chunk, on_update=[]),
                            bass_nofuse=True,
                            engine=inst.engine,
                        )
                        new_insts.append(nop)
                        n_split += 1
                    si.on_wait = keep
                new_insts.append(inst)
            bb.instructions[:] = new_insts
    return n_split


def build_program():
    nc = bass.Bass()
    # x panel, host pre-arranged: [128 part, 32 ktile * 2048 m] bf16
    xp = nc.declare_dram_parameter("xp", [P, NKT * ROWS_PER_CORE], BF16, isOutput=False)
    # W' panel stream, host pre-arranged: [128 part, 16 pass * 32 kt * 256 o]
    wp = nc.declare_dram_parameter("wp", [P, NPASS * NKT * OPW], BF16, isOutput=False)
    outT = nc.declare_dram_parameter("outT", [D, ROWS_PER_CORE], F32, isOutput=True)

    PASS_W = NKT * OPW          # 8192 cols per pass panel
    QW = PASS_W // NWQ          # 2048 cols per quarter panel (8 k-tiles)
    KQ = NKT // NWQ             # 8 k-tiles per quarter panel

    with tile.TileContext(nc) as tc:
        with (
            tc.tile_pool(name="xpool", bufs=1) as xpool,
            tc.tile_pool(name="wpool", bufs=1) as wpool,
            tc.tile_pool(name="opool", bufs=8) as opool,
            tc.tile_pool(name="ppool", bufs=8, space="PSUM") as ppool,
        ):
            # HAM warmup: ~10 junk matmuls on memset tiles run while the
            # first x/W DMAs are in flight, so real matmuls start at 8/8
            if WARMUP:
                jt = xpool.tile([P, MC], BF16, tag="jt", name="jt")
                nc.vector.memset(jt[:], 0)
                jp = ppool.tile([P, MC], F32, tag="acc", name="junkp")
                for i in range(12):
                    nc.tensor.matmul(jp[:], jt[:, :P], jt[:], start=True, stop=True)

            # resident x: 128 (k-tile, m-chunk) chunks, issued in the
            # consumption order of the o-quad first sweep: m-chunks 0-1
            # for all k first, then m-chunks 2-3. Fine granularity keeps
            # stall quanta far under the 3.4us HAM idle window.
            xts = [None] * (NMC * NKT)
            seq = 0
            for h in range(2):
                for k in range(NKT):
                    for mc in (2 * h, 2 * h + 1):
                        i = NMC * k + mc
                        xt = xpool.tile([P, MC], BF16, tag=f"x{i}", name=f"x{i}")
                        if XQ2:
                            eng = nc.gpsimd if (seq < 8 or seq % 2 == 0) else nc.scalar
                        else:
                            eng = nc.gpsimd
                        eng.dma_start(xt[:], xp[:, i * MC:(i + 1) * MC])
                        xts[i] = xt
                        seq += 1

            def xsl(k, mc):
                return xts[NMC * k + mc][:]

            def drain(p, ps, ot, mcn, idx):
                o0 = p * OPW + ot * P
                otile = opool.tile([P, MC], F32, tag="ot",
                                   name=f"o_{p}_{ot}_{mcn}")
                # out-DMA issued from the same engine as the copy: no
                # cross-engine sem hop, and the Pool/SP queues stay clear
                if idx % 2 == 0:
                    nc.vector.tensor_copy(otile[:], ps[:])
                    eng = nc.sync if OALT else nc.gpsimd
                else:
                    nc.scalar.copy(otile[:], ps[:])
                    eng = nc.scalar if OALT else nc.gpsimd
                eng.dma_start(
                    outT[o0:o0 + P, mcn * MC:(mcn + 1) * MC], otile[:])

            # --- first sweep: o-quad (o 0-511) x m-half groups. 4 o-tiles
            # x 2 m-chunks = 8 PSUM banks; halves the x-stream bandwidth
            # demand while x is still loading, so the PE never outruns HBM.
            QUAD_O = 4 * P              # 512 o columns
            QQW = KQ * QUAD_O           # 4096 cols per quad quarter panel
            wqs = []

            def fetch_quad(q):
                wq = wpool.tile([P, QQW], BF16, tag=f"wqd{q}", name=f"wqd{q}")
                nc.sync.dma_start(wq[:], wp[:, q * QQW:(q + 1) * QQW])
                wqs.append(wq)

            fetch_quad(0)
            for g, mcs in enumerate([(0, 1), (2, 3)]):
                psums = {}
                for ot in range(4):
                    for mcn in mcs:
                        psums[ot, mcn] = ppool.tile(
                            [P, MC], F32, tag="acc", name=f"ps_q_{ot}_{mcn}")
                for k in range(NKT):
                    if g == 0 and k in (2, KQ, 2 * KQ):
                        fetch_quad(len(wqs))
                    wt = wqs[k // KQ]
                    koff = (k % KQ) * QUAD_O
                    for ot in range(4):
                        stat = wt[:, koff + ot * P:koff + (ot + 1) * P]
                        for mcn in mcs:
                            nc.tensor.matmul(
                                psums[ot, mcn][:],
                                stat,
                                xsl(k, mcn),
                                start=(k == 0),
                                stop=(k == NKT - 1),
                            )
                for idx, ((ot, mcn), ps) in enumerate(psums.items()):
                    drain(0, ps, ot, mcn, idx)

            # --- remaining o-pair passes (o 512-4095)
            QUAD_COLS = NKT * QUAD_O    # 16384 cols in the quad region
            for p in range(2, NPASS):
                base = QUAD_COLS + (p - 2) * PASS_W
                wqs = []

                def fetch_wq(q, p=p, base=base):
                    wq = wpool.tile([P, QW], BF16, tag=f"wq{q}", name=f"w{p}_{q}")
                    nc.sync.dma_start(
                        wq[:], wp[:, base + q * QW:base + (q + 1) * QW])
                    wqs.append(wq)

                for q in range(NWQ):
                    fetch_wq(q)

                # last pass runs as two half-passes (m-chunks 0-1, then 2-3)
                # so half the PSUM drain overlaps compute instead of tailing
                mc_groups = ([(0, 1), (2, 3)] if (SPLIT and p == NPASS - 1)
                             else [(0, 1, 2, 3)])
                for g, mcs in enumerate(mc_groups):
                    psums = {}
                    for ot in range(2):
                        for mcn in mcs:
                            psums[ot, mcn] = ppool.tile(
                                [P, MC], F32, tag="acc", name=f"ps_{p}_{ot}_{mcn}")

                    for k in range(NKT):
                        wt = wqs[k // KQ]
                        koff = (k % KQ) * OPW
                        for ot in range(2):
                            stat = wt[:, koff + ot * P:koff + (ot + 1) * P]
                            for mcn in mcs:
                                nc.tensor.matmul(
                                    psums[ot, mcn][:],
                                    stat,
                                    xsl(k, mcn),
                                    start=(k == 0),
                                    stop=(k == NKT - 1),
                                )

                    for idx, ((ot, mcn), ps) in enumerate(psums.items()):
                        drain(p, ps, ot, mcn, idx)

    split_wide_waits(nc)
    return nc


_NC_CACHE = [None]


def kernel(x, weight, lora_A, lora_B):
    from concourse.bass_utils import run_bass_kernel_spmd

    x = np.asarray(x, dtype=np.float32)
    weight = np.asarray(weight, dtype=np.float32)
    lora_A = np.asarray(lora_A, dtype=np.float32)
    lora_B = np.asarray(lora_B, dtype=np.float32)

    # fold LoRA: out = x @ (W.T + 2*B@A)
    W2 = (weight.T + 2.0 * (lora_B @ lora_A)).astype(BF)
    full = W2.reshape(NKT, P, D)                    # [kt, part, o]
    # o-quad region (o 0-511): [part, kt, 512] flat
    quad = full[:, :, :512].transpose(1, 0, 2).reshape(P, NKT * 512)
    # o-pair region (o 512-4095): [part, pass, kt, opw] flat
    pairs = (full[:, :, 512:]
             .reshape(NKT, P, NPASS - 2, OPW)
             .transpose(1, 2, 0, 3)
             .reshape(P, (NPASS - 2) * NKT * OPW))
    w4 = np.ascontiguousarray(np.concatenate([quad, pairs], axis=1))

    x2 = x.reshape(ROWS_TOTAL, D).astype(BF)

    in_maps = []
    for c in range(N_CORES):
        xc = x2[c * ROWS_PER_CORE:(c + 1) * ROWS_PER_CORE]      # [2048 m, 4096 k]
        # [k, m] -> [part, kt, m] -> flat [128, NKT*2048]
        x3 = np.ascontiguousarray(
            xc.T.reshape(NKT, P, ROWS_PER_CORE).transpose(1, 0, 2)
        ).reshape(P, NKT * ROWS_PER_CORE)
        in_maps.append({"xp": x3, "wp": w4})

    if _NC_CACHE[0] is None:
        _NC_CACHE[0] = build_program()
    nc = _NC_CACHE[0]

    res = run_bass_kernel_spmd(nc, in_maps, list(range(N_CORES)))
    out = np.empty((ROWS_TOTAL, D), dtype=np.float32)
    for c in range(N_CORES):
        out[c * ROWS_PER_CORE:(c + 1) * ROWS_PER_CORE] = res.results[c]["outT"].T
    return out.reshape(x.shape)


# revision 25
# speedup vs baseline: 1.0120x; 1.0120x over previous
"""LoRA layer kernel for Trainium2, 8-core data-parallel.

out = x @ W.T + 2.0 * ((x @ B) @ A)
  x: (4, 4096, 4096) f32, W: (4096, 4096), A: (16, 4096), B: (4096, 16)

v2 strategy:
  - Host folds LoRA into the weight: W' = W.T + 2*(B@A)  ([in,out]),
    so the device runs a single pure GEMM out = x @ W'.
  - bf16 operands (same PE rate as fp32r at 1 col/cycle, half the DMA
    traffic; fp32 PSUM accumulation keeps rel err ~3e-3).
  - Rows sharded across 8 cores (2048 rows each). Per core the x panel
    ([4096 K, 2048 M] bf16 = 128 KB/partition) is fully SBUF-resident,
    so W' streams from HBM exactly once (32 MB bf16).
  - W-stationary / x-moving: out.T[o,m] = sum_k W'[k,o]*x[k,m]. Each
    pass covers a 256-wide o-pair: 8 PSUM banks = 2 o-tiles x 4
    m-chunks, accumulated over all 32 k-tiles (K-contiguous keeps the
    PE HAM-warm). 4 consecutive matmuls share one stationary W tile.
  - Output produced transposed ([4096 o, 2048 m] per core); host
    transposes back when gathering.
"""

import sys

if "/opt/trn_rl_repo" not in sys.path:
    sys.path.insert(0, "/opt/trn_rl_repo")

import os

import numpy as np
import ml_dtypes

WARMUP = os.environ.get("K_WARMUP", "1") == "1"
XQ2 = os.environ.get("K_XQ2", "1") == "1"
OALT = os.environ.get("K_OALT", "1") == "1"
SPLIT = os.environ.get("K_SPLIT", "1") == "1"

import concourse.bass as bass
import concourse.mybir as mybir
import concourse.tile as tile

N_CORES = 8
D = 4096
RANK = 16
ROWS_TOTAL = 4 * 4096                   # 16384
ROWS_PER_CORE = ROWS_TOTAL // N_CORES   # 2048
P = 128
NKT = D // P                            # 32 k-tiles
NPASS = 16                              # o-pairs of 256
OPW = 256                               # o columns per pass
MC = 512                                # moving m-chunk width
NMC = ROWS_PER_CORE // MC               # 4 m-chunks
NWQ = 4                                 # W quarter-panels per pass

F32 = mybir.dt.float32
BF16 = mybir.dt.bfloat16
BF = ml_dtypes.bfloat16


def split_wide_waits(nc, max_waits=1):
    """walrus in this container rejects >1 sync wait per instruction;
    move excess waits onto preceding same-engine NoOps."""
    n_split = 0
    for f in nc.m.functions:
        for bb in f.blocks:
            new_insts = []
            for inst in bb.instructions:
                si = getattr(inst, "sync_info", None)
                if si is not None and si.on_wait and len(si.on_wait) > max_waits:
                    waits = list(si.on_wait)
                    keep = waits[-max_waits:]
                    extra = waits[:-max_waits]
                    for i in range(0, len(extra), max_waits):
                        chunk = extra[i:i + max_waits]
                        nop = mybir.InstNoOp(
                            name=f"{inst.name}_wsplit{i}",
                            sync_info=mybir.SyncInfo(on_wait=chunk, on_update=[]),
                            bass_nofuse=True,
                            engine=inst.engine,
                        )
                        new_insts.append(nop)
                        n_split += 1
                    si.on_wait = keep
                new_insts.append(inst)
            bb.instructions[:] = new_insts
    return n_split


def build_program():
    nc = bass.Bass()
    # x panel, host pre-arranged: [128 part, 32 ktile * 2048 m] bf16
    xp = nc.declare_dram_parameter("xp", [P, NKT * ROWS_PER_CORE], BF16, isOutput=False)
    # W' panel stream, host pre-arranged: [128 part, 16 pass * 32 kt * 256 o]
    wp = nc.declare_dram_parameter("wp", [P, NPASS * NKT * OPW], BF16, isOutput=False)
    outT = nc.declare_dram_parameter("outT", [D, ROWS_PER_CORE], F32, isOutput=True)

    PASS_W = NKT * OPW          # 8192 cols per pass panel
    QW = PASS_W // NWQ          # 2048 cols per quarter panel (8 k-tiles)
    KQ = NKT // NWQ             # 8 k-tiles per quarter panel

    with tile.TileContext(nc) as tc:
        with (
            tc.tile_pool(name="xpool", bufs=1) as xpool,
            tc.tile_pool(name="wpool", bufs=1) as wpool,
            tc.tile_pool(name="opool", bufs=8) as opool,
            tc.tile_pool(name="ppool", bufs=8, space="PSUM") as ppool,
        ):
            # HAM warmup: ~10 junk matmuls on memset tiles run while the
            # first x/W DMAs are in flight, so real matmuls start at 8/8
            if WARMUP:
                jt = xpool.tile([P, MC], BF16, tag="jt", name="jt")
                nc.vector.memset(jt[:], 0)
                jp = ppool.tile([P, MC], F32, tag="acc", name="junkp")
                for i in range(12):
                    nc.tensor.matmul(jp[:], jt[:, :P], jt[:], start=True, stop=True)

            # resident x, issued in the consumption order of the o-quad
            # first sweep (m-half 0 of each k-tile first). Even k-tiles:
            # fine 128KB m-chunks on the Pool queue (25ns issue). Odd
            # k-tiles: whole 512KB k-tiles on the ACT queue, whose 667ns
            # per-DMA issue cost would otherwise rate-limit the stream.
            xts = [None] * (NMC * NKT)
            xbig = [None] * NKT
            for h in range(2):
                for k in range(NKT):
                    if k % 2 == 0:
                        for mc in (2 * h, 2 * h + 1):
                            i = NMC * k + mc
                            xt = xpool.tile([P, MC], BF16, tag=f"x{i}",
                                            name=f"x{i}")
                            nc.gpsimd.dma_start(
                                xt[:], xp[:, i * MC:(i + 1) * MC])
                            xts[i] = xt
                    elif h == 0:
                        xb = xpool.tile([P, NMC * MC], BF16, tag=f"xb{k}",
                                        name=f"xb{k}")
                        eng = nc.scalar if XQ2 else nc.gpsimd
                        eng.dma_start(
                            xb[:],
                            xp[:, NMC * k * MC:NMC * (k + 1) * MC])
                        xbig[k] = xb

            def xsl(k, mc):
                if k % 2 == 0:
                    return xts[NMC * k + mc][:]
                return xbig[k][:, mc * MC:(mc + 1) * MC]

            def drain(p, ps, ot, mcn, idx):
                o0 = p * OPW + ot * P
                otile = opool.tile([P, MC], F32, tag="ot",
                                   name=f"o_{p}_{ot}_{mcn}")
                # out-DMA issued from the same engine as the copy: no
                # cross-engine sem hop, and the Pool/SP queues stay clear
                if idx % 2 == 0:
                    nc.vector.tensor_copy(otile[:], ps[:])
                    eng = nc.sync if OALT else nc.gpsimd
                else:
                    nc.scalar.copy(otile[:], ps[:])
                    eng = nc.scalar if OALT else nc.gpsimd
                eng.dma_start(
                    outT[o0:o0 + P, mcn * MC:(mcn + 1) * MC], otile[:])

            # --- first sweep: o-quad (o 0-511) x m-half groups. 4 o-tiles
            # x 2 m-chunks = 8 PSUM banks; halves the x-stream bandwidth
            # demand while x is still loading, so the PE never outruns HBM.
            QUAD_O = 4 * P              # 512 o columns
            QQW = KQ * QUAD_O           # 4096 cols per quad quarter panel
            wqs = []

            def fetch_quad(q):
                wq = wpool.tile([P, QQW], BF16, tag=f"wqd{q}", name=f"wqd{q}")
                nc.sync.dma_start(wq[:], wp[:, q * QQW:(q + 1) * QQW])
                wqs.append(wq)

            fetch_quad(0)
            for g, mcs in enumerate([(0, 1), (2, 3)]):
                psums = {}
                for ot in range(4):
                    for mcn in mcs:
                        psums[ot, mcn] = ppool.tile(
                            [P, MC], F32, tag="acc", name=f"ps_q_{ot}_{mcn}")
                for k in range(NKT):
                    if g == 0 and k in (2, KQ, 2 * KQ):
                        fetch_quad(len(wqs))
                    wt = wqs[k // KQ]
                    koff = (k % KQ) * QUAD_O
                    for ot in range(4):
                        stat = wt[:, koff + ot * P:koff + (ot + 1) * P]
                        for mcn in mcs:
                            nc.tensor.matmul(
                                psums[ot, mcn][:],
                                stat,
                                xsl(k, mcn),
                                start=(k == 0),
                                stop=(k == NKT - 1),
                            )
                for idx, ((ot, mcn), ps) in enumerate(psums.items()):
                    drain(0, ps, ot, mcn, idx)

            # --- remaining o-pair passes (o 512-4095)
            QUAD_COLS = NKT * QUAD_O    # 16384 cols in the quad region
            for p in range(2, NPASS):
                base = QUAD_COLS + (p - 2) * PASS_W
                wqs = []

                def fetch_wq(q, p=p, base=base):
                    wq = wpool.tile([P, QW], BF16, tag=f"wq{q}", name=f"w{p}_{q}")
                    nc.sync.dma_start(
                        wq[:], wp[:, base + q * QW:base + (q + 1) * QW])
                    wqs.append(wq)

                for q in range(NWQ):
                    fetch_wq(q)

                # last pass runs as two half-passes (m-chunks 0-1, then 2-3)
                # so half the PSUM drain overlaps compute instead of tailing
                mc_groups = ([(0, 1), (2, 3)] if (SPLIT and p == NPASS - 1)
                             else [(0, 1, 2, 3)])
                for g, mcs in enumerate(mc_groups):
                    psums = {}
                    for ot in range(2):
                        for mcn in mcs:
                            psums[ot, mcn] = ppool.tile(
                                [P, MC], F32, tag="acc", name=f"ps_{p}_{ot}_{mcn}")

                    for k in range(NKT):
                        wt = wqs[k // KQ]
                        koff = (k % KQ) * OPW
                        for ot in range(2):
                            stat = wt[:, koff + ot * P:koff + (ot + 1) * P]
                            for mcn in mcs:
                                nc.tensor.matmul(
                                    psums[ot, mcn][:],
                                    stat,
                                    xsl(k, mcn),
                                    start=(k == 0),
                                    stop=(k == NKT - 1),
                                )

                    for idx, ((ot, mcn), ps) in enumerate(psums.items()):
                        drain(p, ps, ot, mcn, idx)

    split_wide_waits(nc)
    return nc


_NC_CACHE = [None]


def kernel(x, weight, lora_A, lora_B):
    from concourse.bass_utils import run_bass_kernel_spmd

    x = np.asarray(x, dtype=np.float32)
    weight = np.asarray(weight, dtype=np.float32)
    lora_A = np.asarray(lora_A, dtype=np.float32)
    lora_B = np.asarray(lora_B, dtype=np.float32)

    # fold LoRA: out = x @ (W.T + 2*B@A)
    W2 = (weight.T + 2.0 * (lora_B @ lora_A)).astype(BF)
    full = W2.reshape(NKT, P, D)                    # [kt, part, o]
    # o-quad region (o 0-511): [part, kt, 512] flat
    quad = full[:, :, :512].transpose(1, 0, 2).reshape(P, NKT * 512)
    # o-pair region (o 512-4095): [part, pass, kt, opw] flat
    pairs = (full[:, :, 512:]
             .reshape(NKT, P, NPASS - 2, OPW)
             .transpose(1, 2, 0, 3)
             .reshape(P, (NPASS - 2) * NKT * OPW))
    w4 = np.ascontiguousarray(np.concatenate([quad, pairs], axis=1))

    x2 = x.reshape(ROWS_TOTAL, D).astype(BF)

    in_maps = []
    for c in range(N_CORES):
        xc = x2[c * ROWS_PER_CORE:(c + 1) * ROWS_PER_CORE]      # [2048 m, 4096 k]
        # [k, m] -> [part, kt, m] -> flat [128, NKT*2048]
        x3 = np.ascontiguousarray(
            xc.T.reshape(NKT, P, ROWS_PER_CORE).transpose(1, 0, 2)
        ).reshape(P, NKT * ROWS_PER_CORE)
        in_maps.append({"xp": x3, "wp": w4})

    if _NC_CACHE[0] is None:
        _NC_CACHE[0] = build_program()
    nc = _NC_CACHE[0]

    res = run_bass_kernel_spmd(nc, in_maps, list(range(N_CORES)))
    out = np.empty((ROWS_TOTAL, D), dtype=np.float32)
    for c in range(N_CORES):
        out[c * ROWS_PER_CORE:(c + 1) * ROWS_PER_CORE] = res.results[c]["outT"].T
    return out.reshape(x.shape)
